# revision 43
# baseline (speedup 1.0000x reference)
"""Fused multi-head attention with Transformer-XL relative position bias.

8-way head-parallel Bass/Tile kernel for TRN2 (one core per head).

Key trick: the relative-position band term band[q,k] = q_q . emb_{q-k} is a
matmul, because sin(w(q-k)+p) = sin(wq+p)cos(wk) - cos(wq+p)sin(wk).  With
t = q @ positional^T (per-head [q,64]), u = [t*sinQ, -t*cosQ] ([q,128]) and
c = [cosK, sinK] ([k,128]) we have band = u @ c^T exactly.  So the logits are
one matmul with contraction 64(qk) + 128(band), computed directly in
transposed [k, q] layout - softmax denominators come from a ones-column in
the AV matmul, and no transposes of the probability matrix are needed.

Host<->device traffic over the axon tunnel (~35MB/s, ~70ms fixed RTT per
round trip) is the wall-clock bottleneck -- the device kernel itself runs in
a few ms.  Transport design:
  - ALL per-call inputs ship as ONE sharded fp16 blob (one transfer): x is
    (b,s)-block-sharded 1/8 per core and AllGathered on device; qkv/positional
    are head-sharded; out_w is row-sharded (the head dims each core owns)
  - input-independent constants (csq/csk/mask/identity/ones) are committed to
    device once and reused across calls; the jitted executable is cached;
    dummy output operands are device-resident (no host-shipped zeros)
  - the output Dense is row-parallel (per the head shard) with an on-device
    ReduceScatter(add); each core bias-adds + transposes its seq-slice to
    (s,x)-major, the slices are AllGathered so every core holds the final
    [B*S, X], which is emitted int8-quantized with a dynamic global absmax
    scale (AllReduce-max) tucked into one extra row.  The host fetches a
    single replicated int8 array and dequantizes: ~2.1MB down, ~5.4MB up.

Per core (head h = core index), per batch b:
  xg = AllGather(x shard)                [B*S, X] f16 in dram
  xT = x[b]^T (PE transposes)            [512, 2048] f32r
  qT|kT = wqk^T @ xT (+q_bias on q)      [64, 2048] each
  tT = posT @ qT; u = [t*sinQ; -t*cosQ]  [128, 2048]
  for each q-chunk of 512, k-tile of 128 (causal only):
    sT += kT-slice^T-matmul + csk-slice/u matmul   [128k, 512q] PSUM
    pT = exp(0.125 * sT + mask)                     (ACT, writes SBUF)
    oT += v_aug[kt]^T @ pT                          [65, 512] PSUM (row0=denom)
  oT_norm = oT[1:65] * (1/oT[0])                    [64, 512] per q-chunk
  outT_part[b] = outw_rows^T @ oT_norm              [512, 2048] partial Dense
  ReduceScatter(add) -> this core's 512 (b,s)-rows; + out_b; PE-transpose;
  absmax -> AllReduce(max) -> int8 quantize -> AllGather -> out_q + scale row
Host: out = int8 * (gmax/127), reshape to [2, 2048, 512].
"""

import numpy as np

B, S, X = 2, 2048, 512
HEADS, HD = 8, 64
FREQS, MAX_PERIOD = 64, 10000
N_CORES = 8
QS = S // N_CORES  # 256 per-core output sequence slice

_CACHE = {}

# Packed per-core input blob layout (f16 elements): all per-call inputs ship
# as ONE sharded array to amortize per-transfer tunnel overhead.
_PK_LAYOUT = [
    ("xs", B * QS * X),
    ("wqk", X * 128),
    ("wv", X * HD),
    ("posT", HD * FREQS),
    ("qbias", HD),
    ("outw", HD * X),
    ("outb", X),
]
_PK_OFF = {}
_o = 0
for _n, _s in _PK_LAYOUT:
    _PK_OFF[_n] = _o
    _o += _s
NPK = _o


def _host_constants():
    idx = np.arange(FREQS)
    freq = np.pi * (2 / MAX_PERIOD) ** (idx // 2 / (FREQS // 2 - 1))
    phase = np.pi / 2 * (idx % 2)
    t = np.arange(S)
    arg_q = freq[None, :] * t[:, None] + phase[None, :]  # [q, f]
    csq = np.concatenate([np.sin(arg_q), -np.cos(arg_q)], axis=1).T  # [128, S]
    arg_k = freq[None, :] * t[:, None]  # [k, f]
    csk = np.concatenate([np.cos(arg_k), np.sin(arg_k)], axis=1).T  # [128, S]
    kl = np.arange(128)[:, None]
    jl = np.arange(128)[None, :]
    maskadd = np.where(jl >= kl, 0.0, -1e5)  # [128 k, 128 q]
    ident = np.eye(128)
    return (csq.astype(np.float32), csk.astype(np.float32),
            maskadd.astype(np.float32), ident.astype(np.float32))


def _build():
    import concourse.mybir as mybir
    from concourse import bacc, bass_isa
    from concourse.tile import TileContext

    f32 = mybir.dt.float32
    f32r = mybir.dt.float32r
    f16 = mybir.dt.float16
    i8 = mybir.dt.int8

    nc = bacc.Bacc(num_devices=N_CORES, trn_type="TRN2")

    pk = nc.declare_dram_parameter("pk", [NPK, 1], f16, isOutput=False)
    csq = nc.declare_dram_parameter("csq", [128, S], f32, isOutput=False)
    csk = nc.declare_dram_parameter("csk", [128, S], f32, isOutput=False)
    maskadd = nc.declare_dram_parameter("maskadd", [128, 128], f32, isOutput=False)
    ident = nc.declare_dram_parameter("ident", [128, 128], f32, isOutput=False)
    ones = nc.declare_dram_parameter("ones", [128, 1], f32, isOutput=False)
    # int8 output + dynamic scale: row B*S carries the global absmax (f32
    # bitcast into 4 bytes); host dequantizes with gmax/127.
    out_q = nc.declare_dram_parameter("out_q", [B * S + 1, X], i8, isOutput=True)

    NXS = B * QS * X  # xs elements per core
    SC = B * S // N_CORES  # 512 (b,s)-major output rows per core
    xs_l = nc.dram_tensor("xs_l", [NXS, 1], f16)
    xg = nc.dram_tensor("xg", [N_CORES * NXS, 1], f16, addr_space="Shared")
    ag_in = nc.dram_tensor("ag_in", [SC, X], i8)
    ag_out = nc.dram_tensor("ag_out", [B * S, X], i8, addr_space="Shared")
    mx_l = nc.dram_tensor("mx_l", [1, 1], f32)
    mx_g = nc.dram_tensor("mx_g", [1, 1], f32)

    def pk_s(name, lo, n):
        ofs = _PK_OFF[name] + lo
        return pk[ofs:ofs + n, :]
    # ReduceScatter chunk g covers rows [512g, 512g+512) of the (b,s)-major
    # output, i.e. batch g//4, seq 512*(g%4):+512 — core g ends up with them.
    rs_in = nc.dram_tensor("rs_in", [N_CORES, X, SC], f32)
    rs_out = nc.dram_tensor("rs_out", [X, SC], f32)

    NQT = S // 128   # 16 q/k tiles of 128
    NQC = S // 512   # 4 q chunks of 512
    NDT = X // 128   # 4 contraction tiles of 128
    GROUPS = [list(range(N_CORES))]

    with TileContext(nc) as tc:
        with tc.tile_pool(name="const", bufs=1) as cpool, \
             tc.tile_pool(name="xnat", bufs=5) as xnpool, \
             tc.tile_pool(name="xt", bufs=1) as xtpool, \
             tc.tile_pool(name="kq", bufs=2) as kqpool, \
             tc.tile_pool(name="vv", bufs=32) as vpool, \
             tc.tile_pool(name="pt", bufs=2) as ptpool, \
             tc.tile_pool(name="sm", bufs=2) as smpool, \
             tc.tile_pool(name="oc", bufs=8) as ocpool, \
             tc.tile_pool(name="ot", bufs=4) as otpool, \
             tc.tile_pool(name="ps512", bufs=4, space="PSUM") as ps512, \
             tc.tile_pool(name="pso", bufs=2, space="PSUM") as pso:

            # ---- gather the sequence-sharded input across cores ----
            # (collectives cannot read IO tensors; stage through local dram)
            nc.sync.dma_start(out=xs_l[:], in_=pk_s("xs", 0, NXS))
            tc.strict_bb_all_engine_barrier()
            nc.gpsimd.collective_compute(
                "AllGather", mybir.AluOpType.bypass,
                replica_groups=GROUPS, ins=[xs_l[:]], outs=[xg[:]])
            tc.strict_bb_all_engine_barrier()

            # ---- constants to SBUF ----
            csq_sb = cpool.tile([128, S], f32)
            nc.sync.dma_start(out=csq_sb[:], in_=csq[:])
            csk_sb = cpool.tile([128, S], f32r)
            nc.sync.dma_start(out=csk_sb[:], in_=csk[:].bitcast(f32r))
            # fp16 on the wire, converted to f32r on device
            wqk_h = cpool.tile([128, NDT, 128], f16)
            wqk_sb = cpool.tile([128, NDT, 128], f32r)
            for dt in range(NDT):
                nc.sync.dma_start(
                    out=wqk_h[:, dt, :],
                    in_=pk_s("wqk", dt * 128 * 128, 128 * 128).rearrange(
                        "(p f) o -> p (f o)", p=128))
                nc.vector.tensor_copy(wqk_sb[:, dt, :], wqk_h[:, dt, :])
            wv_h = cpool.tile([128, NDT, HD], f16)
            wv_sb = cpool.tile([128, NDT, HD], f32r)
            for dt in range(NDT):
                nc.sync.dma_start(
                    out=wv_h[:, dt, :],
                    in_=pk_s("wv", dt * 128 * HD, 128 * HD).rearrange(
                        "(p f) o -> p (f o)", p=128))
                nc.vector.tensor_copy(wv_sb[:, dt, :], wv_h[:, dt, :])
            posT_h = cpool.tile([HD, FREQS], f16)
            nc.sync.dma_start(out=posT_h[:],
                              in_=pk_s("posT", 0, HD * FREQS).rearrange(
                                  "(p f) o -> p (f o)", p=HD))
            posT_sb = cpool.tile([HD, FREQS], f32r)
            nc.vector.tensor_copy(posT_sb[:], posT_h[:])
            qb_h = cpool.tile([HD, 1], f16)
            nc.sync.dma_start(out=qb_h[:], in_=pk_s("qbias", 0, HD))
            qbias_sb = cpool.tile([HD, 1], f32)
            nc.vector.tensor_copy(qbias_sb[:], qb_h[:])
            outw_h = cpool.tile([HD, X], f16)
            nc.sync.dma_start(out=outw_h[:],
                              in_=pk_s("outw", 0, HD * X).rearrange(
                                  "(p f) o -> p (f o)", p=HD))
            outw_sb = cpool.tile([HD, X], f32r)
            nc.vector.tensor_copy(outw_sb[:], outw_h[:])
            ob_h = cpool.tile([128, NDT], f16)
            nc.sync.dma_start(out=ob_h[:],
                              in_=pk_s("outb", 0, X).rearrange(
                                  "(t p) o -> p (t o)", p=128))
            outb_sb = cpool.tile([128, NDT], f32)
            nc.vector.tensor_copy(outb_sb[:], ob_h[:])
            maskadd_sb = cpool.tile([128, 128], f32)
            nc.sync.dma_start(out=maskadd_sb[:], in_=maskadd[:])
            ident_sb = cpool.tile([128, 128], f32)
            nc.sync.dma_start(out=ident_sb[:], in_=ident[:])
            ident_h = cpool.tile([128, 128], f16)
            nc.vector.tensor_copy(ident_h[:], ident_sb[:])

            for b in range(B):
                # ---- S1: xT = x[b]^T ----
                xt_sb = [xtpool.tile([128, S], f32r, tag=f"xt{dt}", name=f"xt{dt}_{b}") for dt in range(NDT)]
                for g in range(4):  # groups of 4 s-tiles
                    xns = []
                    for si in range(4):
                        st = 4 * g + si
                        xn = xnpool.tile([128, X], f16, name=f"xn{b}_{g}_{si}", tag="xn")
                        # xg is the (b,s)-major flat x: shard g holds rows
                        # [512g, 512g+512) of x.reshape(B*S, X)
                        base = (4 * b + st // 4) * (512 * X) + 128 * (st % 4) * X
                        nc.sync.dma_start(
                            out=xn[:],
                            in_=xg[base:base + 128 * X, :].rearrange(
                                "(p f) o -> p (f o)", p=128))
                        xns.append(xn)
                    for dt in range(NDT):
                        tp = ps512.tile([128, 512], f16, name=f"tp{b}_{g}_{dt}", tag="tps", bufs=2)
                        for si in range(4):
                            nc.tensor.transpose(
                                tp[:, 128 * si:128 * si + 128],
                                xns[si][:, 128 * dt:128 * dt + 128],
                                ident_h[:])
                        nc.vector.tensor_copy(xt_sb[dt][:, 512 * g:512 * g + 512], tp[:])

                # ---- S2: projections ----
                qT_sb = kqpool.tile([HD, S], f32r, tag="qT")
                kT_sb = kqpool.tile([HD, S], f32r, tag="kT")
                for ch in range(NQC):
                    ps = ps512.tile([128, 512], f32, tag='ps', bufs=2)
                    for dt in range(NDT):
                        nc.tensor.matmul(ps[:], wqk_sb[:, dt, :],
                                         xt_sb[dt][:, 512 * ch:512 * ch + 512],
                                         start=(dt == 0), stop=(dt == NDT - 1))
                    nc.scalar.activation(qT_sb[:, 512 * ch:512 * ch + 512], ps[0:HD, :],
                                         mybir.ActivationFunctionType.Identity,
                                         bias=qbias_sb[:, 0:1])
                    nc.vector.tensor_copy(kT_sb[:, 512 * ch:512 * ch + 512], ps[HD:128, :])

                v_sb = []
                for st in range(NQT):
                    vt = vpool.tile([128, HD + 1], f32r, tag="v", name=f"v{b}_{st}")
                    nc.sync.dma_start(out=vt[:, HD:HD + 1], in_=ones[:].bitcast(f32r))
                    ps = ps512.tile([128, 512], f32, tag='ps', bufs=2)
                    for dt in range(NDT):
                        nc.tensor.matmul(ps[:, 0:HD], xt_sb[dt][:, 128 * st:128 * st + 128],
                                         wv_sb[:, dt, :],
                                         start=(dt == 0), stop=(dt == NDT - 1))
                    nc.vector.tensor_copy(vt[:, 0:HD], ps[:, 0:HD])
                    v_sb.append(vt)

                u_sb = kqpool.tile([128, S], f32r, tag="u", bufs=1)
                for ch in range(NQC):
                    ps = ps512.tile([128, 512], f32, tag='ps', bufs=2)
                    nc.tensor.matmul(ps[0:HD, :], posT_sb[:],
                                     qT_sb[:, 512 * ch:512 * ch + 512],
                                     start=True, stop=True)
                    sl = slice(512 * ch, 512 * ch + 512)
                    nc.vector.tensor_mul(u_sb[0:64, sl], ps[0:HD, :], csq_sb[0:64, sl])
                    nc.vector.tensor_mul(u_sb[64:128, sl], ps[0:HD, :], csq_sb[64:128, sl])

                # ---- S3: attention ----
                o_chunks = []
                for qc in range(NQC):
                    qsl = slice(512 * qc, 512 * qc + 512)
                    o_ps = pso.tile([HD + 1, 512], f32)
                    n_kt = 4 * qc + 4
                    for kt in range(n_kt):
                        s_ps = ps512.tile([128, 512], f32, tag='sps', bufs=2)
                        nc.tensor.matmul(s_ps[:], kT_sb[:, 128 * kt:128 * kt + 128],
                                         qT_sb[:, qsl], start=True, stop=False)
                        nc.tensor.matmul(s_ps[:], csk_sb[:, 128 * kt:128 * kt + 128],
                                         u_sb[:, qsl], start=False, stop=True)
                        m = kt - 4 * qc
                        if m > 0:
                            nc.vector.tensor_scalar_add(s_ps[:, 0:128 * m],
                                                        s_ps[:, 0:128 * m], -1e5)
                        if m >= 0:
                            msl = slice(128 * m, 128 * m + 128)
                            nc.vector.tensor_add(s_ps[:, msl], s_ps[:, msl], maskadd_sb[:])
                        p_sb = ptpool.tile([128, 512], f32r, tag="pT")
                        nc.scalar.activation(p_sb[:], s_ps[:],
                                             mybir.ActivationFunctionType.Exp,
                                             scale=0.125)
                        nc.tensor.matmul(o_ps[:], v_sb[kt][:], p_sb[:],
                                         start=(kt == 0), stop=(kt == n_kt - 1))
                    recip = smpool.tile([1, 512], f32, tag="recip")
                    nc.vector.reciprocal(recip[:], o_ps[HD:HD + 1, :])
                    bcast = smpool.tile([HD, 512], f32, tag="bcast")
                    nc.gpsimd.partition_broadcast(bcast[:], recip[:])
                    o_sb = ocpool.tile([HD, 512], f32r, tag="osb", name=f"o{b}_{qc}")
                    nc.vector.tensor_mul(o_sb[:], o_ps[0:HD, :], bcast[:])
                    o_chunks.append(o_sb)

                # ---- S4: row-parallel output Dense partials ----
                for mt in range(NDT):
                    for ch in range(NQC):
                        ps = ps512.tile([128, 512], f32, tag='ps', bufs=2)
                        nc.tensor.matmul(ps[:], outw_sb[:, 128 * mt:128 * mt + 128],
                                         o_chunks[ch][:], start=True, stop=True)
                        o2 = otpool.tile([128, 512], f32, tag="o2")
                        nc.vector.tensor_copy(o2[:], ps[:])
                        nc.sync.dma_start(
                            out=rs_in[4 * b + ch, 128 * mt:128 * mt + 128, :],
                            in_=o2[:])

            # ---- S5: ReduceScatter partials -> this core's seq slice ----
            tc.strict_bb_all_engine_barrier()
            nc.gpsimd.collective_compute(
                "ReduceScatter", mybir.AluOpType.add,
                replica_groups=GROUPS, ins=[rs_in[:]], outs=[rs_out[:]])
            tc.strict_bb_all_engine_barrier()

            # add bias, transpose to (s, x)-major, and find this core's absmax
            agt = [otpool.tile([128, X], f16, tag=f"agt{s2}", bufs=1,
                               name=f"agt{s2}")
                   for s2 in range(4)]
            mxp = otpool.tile([128, NDT], f32, tag="mxp", bufs=1)
            for mt in range(NDT):
                r_sb = otpool.tile([128, SC], f32, tag="rsb")
                nc.sync.dma_start(out=r_sb[:], in_=rs_out[128 * mt:128 * mt + 128, :])
                o3 = otpool.tile([128, SC], f16, tag="o3")
                nc.scalar.activation(o3[:], r_sb[:],
                                     mybir.ActivationFunctionType.Identity,
                                     bias=outb_sb[:, mt:mt + 1])
                nc.vector.tensor_reduce(mxp[:, mt:mt + 1], o3[:],
                                        axis=mybir.AxisListType.X,
                                        op=mybir.AluOpType.max,
                                        apply_absolute_value=True)
                tpp = ps512.tile([128, 512], f16, tag="tps", bufs=2,
                                 name=f"tpp{mt}")
                for s2 in range(4):
                    nc.tensor.transpose(tpp[:, 128 * s2:128 * s2 + 128],
                                        o3[:, 128 * s2:128 * s2 + 128],
                                        ident_h[:])
                for s2 in range(4):
                    nc.vector.tensor_copy(agt[s2][:, 128 * mt:128 * mt + 128],
                                          tpp[:, 128 * s2:128 * s2 + 128])
            # local absmax -> global absmax across cores
            mx1 = otpool.tile([128, 1], f32, tag="mx1", bufs=1)
            nc.vector.tensor_reduce(mx1[:, 0:1], mxp[:],
                                    axis=mybir.AxisListType.X,
                                    op=mybir.AluOpType.max)
            mxr = otpool.tile([128, 1], f32, tag="mxr", bufs=1)
            nc.gpsimd.partition_all_reduce(mxr[:], mx1[:], channels=128,
                                           reduce_op=bass_isa.ReduceOp.max)
            nc.sync.dma_start(out=mx_l[:], in_=mxr[0:1, 0:1])
            tc.strict_bb_all_engine_barrier()
            nc.gpsimd.collective_compute(
                "AllReduce", mybir.AluOpType.max,
                replica_groups=GROUPS, ins=[mx_l[:]], outs=[mx_g[:]])
            tc.strict_bb_all_engine_barrier()
            # scale = 127 / gmax, broadcast to all partitions
            gmax_sb = otpool.tile([1, 1], f32, tag="gmax", bufs=1)
            nc.sync.dma_start(out=gmax_sb[:], in_=mx_g[:])
            rcp = otpool.tile([1, 1], f32, tag="rcp", bufs=1)
            nc.vector.reciprocal(rcp[:], gmax_sb[:])
            nc.vector.tensor_scalar_mul(rcp[:], rcp[:], 127.0)
            scl = otpool.tile([128, 1], f32, tag="scl", bufs=1)
            nc.gpsimd.partition_broadcast(scl[:], rcp[:])
            # quantize this core's slice to int8 and gather everywhere
            for s2 in range(4):
                qt = otpool.tile([128, X], i8, tag="qt")
                nc.scalar.activation(qt[:], agt[s2][:],
                                     mybir.ActivationFunctionType.Identity,
                                     scale=scl[:, 0:1])
                nc.sync.dma_start(out=ag_in[128 * s2:128 * s2 + 128, :], in_=qt[:])
            tc.strict_bb_all_engine_barrier()
            nc.gpsimd.collective_compute(
                "AllGather", mybir.AluOpType.bypass,
                replica_groups=GROUPS, ins=[ag_in[:]], outs=[ag_out[:]])
            tc.strict_bb_all_engine_barrier()
            nc.sync.dma_start(out=out_q[0:B * S, :], in_=ag_out[:])
            nc.sync.dma_start(out=out_q[B * S:B * S + 1, 0:4],
                              in_=gmax_sb[:].bitcast(i8))

    nc.finalize()
    return nc


class _Runner:
    """Cached jitted shard_map executor for the Bass kernel.

    Mirrors bass2jax.run_bass_via_pjrt but (a) builds the jit once, (b) keeps
    input-independent constants committed on device, (c) materializes output
    buffers in-graph instead of shipping zeros from host.
    """

    CONST_NAMES = ("csq", "csk", "maskadd", "ident", "ones")

    def __init__(self):
        import jax
        import jax.numpy as jnp
        from jax.sharding import Mesh, PartitionSpec, NamedSharding
        from jax.experimental.shard_map import shard_map
        import concourse.mybir as mybir
        from concourse.bass2jax import (
            install_neuronx_cc_hook, partition_id_tensor, _bass_exec_p)

        install_neuronx_cc_hook()
        nc = _build()
        self.nc = nc

        partition_name = nc.partition_id_tensor.name if nc.partition_id_tensor else None
        in_names, out_names, out_avals = [], [], []
        for alloc in nc.m.functions[0].allocations:
            if not isinstance(alloc, mybir.MemoryLocationSet):
                continue
            name = alloc.memorylocations[0].name
            if alloc.kind == "ExternalInput":
                if name != partition_name:
                    in_names.append(name)
            elif alloc.kind == "ExternalOutput":
                out_names.append(name)
                out_avals.append(jax.core.ShapedArray(
                    tuple(alloc.tensor_shape), mybir.dt.np(alloc.dtype)))
        self.in_names = in_names
        self.out_names = out_names
        self.out_avals = out_avals
        in_names_all = in_names + out_names + ([partition_name] if partition_name else [])

        def _body(*args):
            operands = list(args)
            if partition_name is not None:
                operands.append(partition_id_tensor())
            outs = _bass_exec_p.bind(
                *operands,
                out_avals=tuple(out_avals),
                in_names=tuple(in_names_all),
                out_names=tuple(out_names),
                lowering_input_output_aliases=(),
                sim_require_finite=True,
                sim_require_nnan=True,
                nc=nc)
            return tuple(outs)

        devices = jax.devices()[:N_CORES]
        assert len(devices) == N_CORES
        mesh = Mesh(np.asarray(devices), ("core",))
        self.sharding = NamedSharding(mesh, PartitionSpec("core"))
        self.rep_sharding = NamedSharding(mesh, PartitionSpec())
        # out_f is identical on every core (device-side AllGather) ->
        # replicated: jax fetches a single contiguous shard.
        in_specs = (PartitionSpec("core"),) * len(in_names) + \
            (PartitionSpec(),) * len(out_names)
        out_specs = (PartitionSpec(),) * len(out_names)
        self.fn = jax.jit(shard_map(
            _body, mesh=mesh, in_specs=in_specs, out_specs=out_specs,
            check_rep=False))

        # Jitted CPU helpers: multithreaded pack (f32 -> f16 blob) and int8
        # dequant -- ~2x faster than single-threaded numpy casts.
        self._cpu = jax.devices("cpu")[0]

        def _pack(x, qkv, q_bias, positional, out_w, out_b):
            f16 = jnp.float16
            xs = x.reshape(N_CORES, -1).astype(f16)
            wqk = qkv[:, 0:2].transpose(2, 0, 1, 3).reshape(N_CORES, -1).astype(f16)
            wv = qkv[:, 2].transpose(1, 0, 2).reshape(N_CORES, -1).astype(f16)
            posT = positional.transpose(1, 2, 0).reshape(N_CORES, -1).astype(f16)
            qb = q_bias.astype(f16)
            ow = out_w.reshape(N_CORES, -1).astype(f16)
            ob = jnp.broadcast_to(out_b.astype(f16)[None, :], (N_CORES, X))
            return jnp.concatenate([xs, wqk, wv, posT, qb, ow, ob], axis=1)

        def _dq(body, scale):
            return (body.astype(jnp.float32) * scale).reshape(B, S, X)

        with jax.default_device(self._cpu):
            self._pack_fn = jax.jit(_pack)
            self._dq_fn = jax.jit(_dq)

        # Commit input-independent constants to device once.
        csq, csk, maskadd, ident = _host_constants()
        const_global = {
            "csq": np.broadcast_to(csq, (N_CORES,) + csq.shape).reshape(N_CORES * 128, S),
            "csk": np.broadcast_to(csk, (N_CORES,) + csk.shape).reshape(N_CORES * 128, S),
            "maskadd": np.broadcast_to(maskadd, (N_CORES, 128, 128)).reshape(N_CORES * 128, 128),
            "ident": np.broadcast_to(ident, (N_CORES, 128, 128)).reshape(N_CORES * 128, 128),
            "ones": np.ones((N_CORES * 128, 1), np.float32),
        }
        import jax as _jax
        self.const_dev = {
            k: _jax.device_put(np.ascontiguousarray(v), self.sharding)
            for k, v in const_global.items()}
        # Dummy output-operand buffers, committed once (the kernel fully
        # overwrites every output, so their contents are irrelevant).
        self.zero_dev = [
            _jax.device_put(np.zeros(a.shape, a.dtype), self.rep_sharding)
            for a in out_avals]
        _jax.block_until_ready(list(self.const_dev.values()) + self.zero_dev)

    def __call__(self, named_globals):
        args = []
        for name in self.in_names:
            if name in self.const_dev:
                args.append(self.const_dev[name])
            else:
                args.append(named_globals[name])
        args.extend(self.zero_dev)
        outs = self.fn(*args)
        return dict(zip(self.out_names, (np.asarray(o) for o in outs)))


def _get_runner():
    if "runner" not in _CACHE:
        _CACHE["runner"] = _Runner()
    return _CACHE["runner"]


def kernel(x, qkv, q_bias, positional, out_w, out_b, _want_results=False, _trace=False):
    x = np.asarray(x, dtype=np.float32)
    qkv = np.asarray(qkv, dtype=np.float32)
    q_bias = np.asarray(q_bias, dtype=np.float32)
    positional = np.asarray(positional, dtype=np.float32)
    out_w = np.asarray(out_w, dtype=np.float32)
    out_b = np.asarray(out_b, dtype=np.float32)

    runner = _get_runner()
    import jax

    # One packed f16 blob per core (core c == head c == sequence slice c);
    # region order matches _PK_LAYOUT.
    with jax.default_device(runner._cpu):
        blob = np.asarray(
            runner._pack_fn(x, qkv, q_bias, positional, out_w, out_b))

    res = runner({"pk": blob.reshape(N_CORES * NPK, 1)})
    a = res["out_q"]
    gmax = a[B * S, 0:4].copy().view(np.float32)[0]
    with jax.default_device(runner._cpu):
        out = np.asarray(runner._dq_fn(a[:B * S], np.float32(gmax / 127.0)))
    if _want_results:
        class _R:
            exec_time_ns = None
            per_core_scope_times = None
            instructions_and_trace = None
        return out, _R()
    return out


# revision 48
# speedup vs baseline: 1.0854x; 1.0854x over previous
"""Fused multi-head attention with Transformer-XL relative position bias.

8-way head-parallel Bass/Tile kernel for TRN2 (one core per head).

Key trick: the relative-position band term band[q,k] = q_q . emb_{q-k} is a
matmul, because sin(w(q-k)+p) = sin(wq+p)cos(wk) - cos(wq+p)sin(wk).  With
t = q @ positional^T (per-head [q,64]), u = [t*sinQ, -t*cosQ] ([q,128]) and
c = [cosK, sinK] ([k,128]) we have band = u @ c^T exactly.  So the logits are
one matmul with contraction 64(qk) + 128(band), computed directly in
transposed [k, q] layout - softmax denominators come from a ones-column in
the AV matmul, and no transposes of the probability matrix are needed.

Host<->device traffic over the axon tunnel (~35MB/s, ~70ms fixed RTT per
round trip) is the wall-clock bottleneck -- the device kernel itself runs in
a few ms.  Transport design:
  - ALL per-call inputs ship as ONE sharded fp16 blob (one transfer): x is
    (b,s)-block-sharded 1/8 per core and AllGathered on device; qkv/positional
    are head-sharded; out_w is row-sharded (the head dims each core owns)
  - input-independent constants (csq/csk/mask/identity/ones) are committed to
    device once and reused across calls; the jitted executable is cached;
    dummy output operands are device-resident (no host-shipped zeros)
  - the output Dense is row-parallel (per the head shard) with an on-device
    ReduceScatter(add); each core bias-adds + transposes its seq-slice to
    (s,x)-major, the slices are AllGathered so every core holds the final
    [B*S, X], which is emitted int8-quantized with a dynamic global absmax
    scale (AllReduce-max) tucked into one extra row.  The host fetches a
    single replicated int8 array and dequantizes: ~2.1MB down, ~5.4MB up.

Per core (head h = core index), per batch b:
  xg = AllGather(x shard)                [B*S, X] f16 in dram
  xT = x[b]^T (PE transposes)            [512, 2048] f32r
  qT|kT = wqk^T @ xT (+q_bias on q)      [64, 2048] each
  tT = posT @ qT; u = [t*sinQ; -t*cosQ]  [128, 2048]
  for each q-chunk of 512, k-tile of 128 (causal only):
    sT += kT-slice^T-matmul + csk-slice/u matmul   [128k, 512q] PSUM
    pT = exp(0.125 * sT + mask)                     (ACT, writes SBUF)
    oT += v_aug[kt]^T @ pT                          [65, 512] PSUM (row0=denom)
  oT_norm = oT[1:65] * (1/oT[0])                    [64, 512] per q-chunk
  outT_part[b] = outw_rows^T @ oT_norm              [512, 2048] partial Dense
  ReduceScatter(add) -> this core's 512 (b,s)-rows; + out_b; PE-transpose;
  absmax -> AllReduce(max) -> int8 quantize -> AllGather -> out_q + scale row
Host: out = int8 * (gmax/127), reshape to [2, 2048, 512].
"""

import numpy as np

B, S, X = 2, 2048, 512
HEADS, HD = 8, 64
FREQS, MAX_PERIOD = 64, 10000
N_CORES = 8
QS = S // N_CORES  # 256 per-core output sequence slice

_CACHE = {}

# Packed per-core input blob layout (f16 slots): all per-call inputs ship
# as ONE sharded array to amortize per-transfer tunnel overhead.  x ships
# 12-bit quantized (hi-byte plane + packed-nibble plane riding in f16 slots,
# plus the f16 quantization step); weights ship as f16 values.
NXS = B * QS * X  # x elements per core (262144)
_PK_LAYOUT = [
    ("xhi", NXS // 2),    # int8 hi plane (q >> 4), 2 bytes/slot
    ("xnib", NXS // 4),   # packed low nibbles (even value low, odd high)
    ("xstep", 64),        # slot 0 = f16 quantization step, rest pad
    ("wqk", X * 128),
    ("wv", X * HD),
    ("posT", HD * FREQS),
    ("qbias", HD),
    ("outw", HD * X),
    ("outb", X),
]
_PK_OFF = {}
_o = 0
for _n, _s in _PK_LAYOUT:
    _PK_OFF[_n] = _o
    _o += _s
NPK = _o


def _host_constants():
    idx = np.arange(FREQS)
    freq = np.pi * (2 / MAX_PERIOD) ** (idx // 2 / (FREQS // 2 - 1))
    phase = np.pi / 2 * (idx % 2)
    t = np.arange(S)
    arg_q = freq[None, :] * t[:, None] + phase[None, :]  # [q, f]
    csq = np.concatenate([np.sin(arg_q), -np.cos(arg_q)], axis=1).T  # [128, S]
    arg_k = freq[None, :] * t[:, None]  # [k, f]
    csk = np.concatenate([np.cos(arg_k), np.sin(arg_k)], axis=1).T  # [128, S]
    kl = np.arange(128)[:, None]
    jl = np.arange(128)[None, :]
    maskadd = np.where(jl >= kl, 0.0, -1e5)  # [128 k, 128 q]
    ident = np.eye(128)
    return (csq.astype(np.float32), csk.astype(np.float32),
            maskadd.astype(np.float32), ident.astype(np.float32))


def _build():
    import concourse.mybir as mybir
    from concourse import bacc, bass_isa
    from concourse.tile import TileContext

    f32 = mybir.dt.float32
    f32r = mybir.dt.float32r
    f16 = mybir.dt.float16
    i8 = mybir.dt.int8
    u8 = mybir.dt.uint8

    nc = bacc.Bacc(num_devices=N_CORES, trn_type="TRN2")

    pk = nc.declare_dram_parameter("pk", [NPK, 1], f16, isOutput=False)
    csq = nc.declare_dram_parameter("csq", [128, S], f32, isOutput=False)
    csk = nc.declare_dram_parameter("csk", [128, S], f32, isOutput=False)
    maskadd = nc.declare_dram_parameter("maskadd", [128, 128], f32, isOutput=False)
    ident = nc.declare_dram_parameter("ident", [128, 128], f32, isOutput=False)
    ones = nc.declare_dram_parameter("ones", [128, 1], f32, isOutput=False)
    # int8 output + dynamic scale: row B*S carries the global absmax (f32
    # bitcast into 4 bytes); host dequantizes with gmax/127.
    out_q = nc.declare_dram_parameter("out_q", [B * S + 1, X], i8, isOutput=True)

    SC = B * S // N_CORES  # 512 (b,s)-major output rows per core
    xs_l = nc.dram_tensor("xs_l", [NXS, 1], f16)
    xg = nc.dram_tensor("xg", [N_CORES * NXS, 1], f16, addr_space="Shared")
    ag_in = nc.dram_tensor("ag_in", [SC, X], i8)
    ag_out = nc.dram_tensor("ag_out", [B * S, X], i8, addr_space="Shared")
    mx_l = nc.dram_tensor("mx_l", [1, 1], f32)
    mx_g = nc.dram_tensor("mx_g", [1, 1], f32)

    def pk_s(name, lo, n):
        ofs = _PK_OFF[name] + lo
        return pk[ofs:ofs + n, :]
    # ReduceScatter chunk g covers rows [512g, 512g+512) of the (b,s)-major
    # output, i.e. batch g//4, seq 512*(g%4):+512 — core g ends up with them.
    rs_in = nc.dram_tensor("rs_in", [N_CORES, X, SC], f32)
    rs_out = nc.dram_tensor("rs_out", [X, SC], f32)

    NQT = S // 128   # 16 q/k tiles of 128
    NQC = S // 512   # 4 q chunks of 512
    NDT = X // 128   # 4 contraction tiles of 128
    GROUPS = [list(range(N_CORES))]

    with TileContext(nc) as tc:
        with tc.tile_pool(name="const", bufs=1) as cpool, \
             tc.tile_pool(name="xnat", bufs=5) as xnpool, \
             tc.tile_pool(name="xt", bufs=1) as xtpool, \
             tc.tile_pool(name="kq", bufs=2) as kqpool, \
             tc.tile_pool(name="vv", bufs=32) as vpool, \
             tc.tile_pool(name="pt", bufs=2) as ptpool, \
             tc.tile_pool(name="sm", bufs=2) as smpool, \
             tc.tile_pool(name="oc", bufs=8) as ocpool, \
             tc.tile_pool(name="ot", bufs=4) as otpool, \
             tc.tile_pool(name="ps512", bufs=4, space="PSUM") as ps512, \
             tc.tile_pool(name="pso", bufs=2, space="PSUM") as pso:

            # ---- unpack this core's 12-bit x shard to f16, then gather ----
            # value v: hi byte v of xhi (q>>4), nibble of byte v//2 of xnib
            # (v even -> low).  x = (hi*16 + lo) * step.  Row p of the [128,
            # 2048] tiles covers flat values [2048p, 2048(p+1)).
            xhi_sl = cpool.tile([128, NXS // 256], f16)  # [128,1024] slots
            nc.sync.dma_start(out=xhi_sl[:],
                              in_=pk_s("xhi", 0, NXS // 2).rearrange(
                                  "(p f) o -> p (f o)", p=128))
            xnib_sl = cpool.tile([128, NXS // 512], f16)  # [128,512] slots
            nc.sync.dma_start(out=xnib_sl[:],
                              in_=pk_s("xnib", 0, NXS // 4).rearrange(
                                  "(p f) o -> p (f o)", p=128))
            xst_h = cpool.tile([1, 1], f16)
            nc.sync.dma_start(out=xst_h[:], in_=pk_s("xstep", 0, 1))
            xst_f = cpool.tile([1, 1], f32)
            nc.vector.tensor_copy(xst_f[:], xst_h[:])
            xst_bc = cpool.tile([128, 1], f32)
            nc.gpsimd.partition_broadcast(xst_bc[:], xst_f[:])
            hi_f = cpool.tile([128, 2048], f16)
            nc.vector.tensor_copy(hi_f[:], xhi_sl[:].bitcast(i8))
            lo_u8 = cpool.tile([128, 2048], u8)
            lo_v = lo_u8[:].rearrange("p (f two) -> p f two", two=2)
            nib_u8 = xnib_sl[:].bitcast(u8)  # [128, 1024]
            nc.vector.tensor_scalar(lo_v[:, :, 0:1], nib_u8, 15, None,
                                    op0=mybir.AluOpType.bitwise_and)
            nc.vector.tensor_scalar(lo_v[:, :, 1:2], nib_u8, 4, None,
                                    op0=mybir.AluOpType.logical_shift_right)
            lo_f = cpool.tile([128, 2048], f16)
            nc.vector.tensor_copy(lo_f[:], lo_u8[:])
            q_f = cpool.tile([128, 2048], f16)
            nc.vector.tensor_scalar_mul(q_f[:], hi_f[:], 16.0)
            nc.vector.tensor_add(q_f[:], q_f[:], lo_f[:])
            xs_f = cpool.tile([128, 2048], f16)
            nc.scalar.activation(xs_f[:], q_f[:],
                                 mybir.ActivationFunctionType.Identity,
                                 scale=xst_bc[:, 0:1])
            nc.sync.dma_start(
                out=xs_l[:].rearrange("(p f) o -> p (f o)", p=128),
                in_=xs_f[:])
            tc.strict_bb_all_engine_barrier()
            nc.gpsimd.collective_compute(
                "AllGather", mybir.AluOpType.bypass,
                replica_groups=GROUPS, ins=[xs_l[:]], outs=[xg[:]])
            tc.strict_bb_all_engine_barrier()

            # ---- constants to SBUF ----
            csq_sb = cpool.tile([128, S], f32)
            nc.sync.dma_start(out=csq_sb[:], in_=csq[:])
            csk_sb = cpool.tile([128, S], f32r)
            nc.sync.dma_start(out=csk_sb[:], in_=csk[:].bitcast(f32r))
            # fp16 on the wire, converted to f32r on device
            wqk_h = cpool.tile([128, NDT, 128], f16)
            wqk_sb = cpool.tile([128, NDT, 128], f32r)
            for dt in range(NDT):
                nc.sync.dma_start(
                    out=wqk_h[:, dt, :],
                    in_=pk_s("wqk", dt * 128 * 128, 128 * 128).rearrange(
                        "(p f) o -> p (f o)", p=128))
                nc.vector.tensor_copy(wqk_sb[:, dt, :], wqk_h[:, dt, :])
            wv_h = cpool.tile([128, NDT, HD], f16)
            wv_sb = cpool.tile([128, NDT, HD], f32r)
            for dt in range(NDT):
                nc.sync.dma_start(
                    out=wv_h[:, dt, :],
                    in_=pk_s("wv", dt * 128 * HD, 128 * HD).rearrange(
                        "(p f) o -> p (f o)", p=128))
                nc.vector.tensor_copy(wv_sb[:, dt, :], wv_h[:, dt, :])
            posT_h = cpool.tile([HD, FREQS], f16)
            nc.sync.dma_start(out=posT_h[:],
                              in_=pk_s("posT", 0, HD * FREQS).rearrange(
                                  "(p f) o -> p (f o)", p=HD))
            posT_sb = cpool.tile([HD, FREQS], f32r)
            nc.vector.tensor_copy(posT_sb[:], posT_h[:])
            qb_h = cpool.tile([HD, 1], f16)
            nc.sync.dma_start(out=qb_h[:], in_=pk_s("qbias", 0, HD))
            qbias_sb = cpool.tile([HD, 1], f32)
            nc.vector.tensor_copy(qbias_sb[:], qb_h[:])
            outw_h = cpool.tile([HD, X], f16)
            nc.sync.dma_start(out=outw_h[:],
                              in_=pk_s("outw", 0, HD * X).rearrange(
                                  "(p f) o -> p (f o)", p=HD))
            outw_sb = cpool.tile([HD, X], f32r)
            nc.vector.tensor_copy(outw_sb[:], outw_h[:])
            ob_h = cpool.tile([128, NDT], f16)
            nc.sync.dma_start(out=ob_h[:],
                              in_=pk_s("outb", 0, X).rearrange(
                                  "(t p) o -> p (t o)", p=128))
            outb_sb = cpool.tile([128, NDT], f32)
            nc.vector.tensor_copy(outb_sb[:], ob_h[:])
            maskadd_sb = cpool.tile([128, 128], f32)
            nc.sync.dma_start(out=maskadd_sb[:], in_=maskadd[:])
            ident_sb = cpool.tile([128, 128], f32)
            nc.sync.dma_start(out=ident_sb[:], in_=ident[:])
            ident_h = cpool.tile([128, 128], f16)
            nc.vector.tensor_copy(ident_h[:], ident_sb[:])

            for b in range(B):
                # ---- S1: xT = x[b]^T ----
                xt_sb = [xtpool.tile([128, S], f32r, tag=f"xt{dt}", name=f"xt{dt}_{b}") for dt in range(NDT)]
                for g in range(4):  # groups of 4 s-tiles
                    xns = []
                    for si in range(4):
                        st = 4 * g + si
                        xn = xnpool.tile([128, X], f16, name=f"xn{b}_{g}_{si}", tag="xn")
                        # xg is the (b,s)-major flat x: shard g holds rows
                        # [512g, 512g+512) of x.reshape(B*S, X)
                        base = (4 * b + st // 4) * (512 * X) + 128 * (st % 4) * X
                        nc.sync.dma_start(
                            out=xn[:],
                            in_=xg[base:base + 128 * X, :].rearrange(
                                "(p f) o -> p (f o)", p=128))
                        xns.append(xn)
                    for dt in range(NDT):
                        tp = ps512.tile([128, 512], f16, name=f"tp{b}_{g}_{dt}", tag="tps", bufs=2)
                        for si in range(4):
                            nc.tensor.transpose(
                                tp[:, 128 * si:128 * si + 128],
                                xns[si][:, 128 * dt:128 * dt + 128],
                                ident_h[:])
                        nc.vector.tensor_copy(xt_sb[dt][:, 512 * g:512 * g + 512], tp[:])

                # ---- S2: projections ----
                qT_sb = kqpool.tile([HD, S], f32r, tag="qT")
                kT_sb = kqpool.tile([HD, S], f32r, tag="kT")
                for ch in range(NQC):
                    ps = ps512.tile([128, 512], f32, tag='ps', bufs=2)
                    for dt in range(NDT):
                        nc.tensor.matmul(ps[:], wqk_sb[:, dt, :],
                                         xt_sb[dt][:, 512 * ch:512 * ch + 512],
                                         start=(dt == 0), stop=(dt == NDT - 1))
                    nc.scalar.activation(qT_sb[:, 512 * ch:512 * ch + 512], ps[0:HD, :],
                                         mybir.ActivationFunctionType.Identity,
                                         bias=qbias_sb[:, 0:1])
                    nc.vector.tensor_copy(kT_sb[:, 512 * ch:512 * ch + 512], ps[HD:128, :])

                v_sb = []
                for st in range(NQT):
                    vt = vpool.tile([128, HD + 1], f32r, tag="v", name=f"v{b}_{st}")
                    nc.sync.dma_start(out=vt[:, HD:HD + 1], in_=ones[:].bitcast(f32r))
                    ps = ps512.tile([128, 512], f32, tag='ps', bufs=2)
                    for dt in range(NDT):
                        nc.tensor.matmul(ps[:, 0:HD], xt_sb[dt][:, 128 * st:128 * st + 128],
                                         wv_sb[:, dt, :],
                                         start=(dt == 0), stop=(dt == NDT - 1))
                    nc.vector.tensor_copy(vt[:, 0:HD], ps[:, 0:HD])
                    v_sb.append(vt)

                u_sb = kqpool.tile([128, S], f32r, tag="u", bufs=1)
                for ch in range(NQC):
                    ps = ps512.tile([128, 512], f32, tag='ps', bufs=2)
                    nc.tensor.matmul(ps[0:HD, :], posT_sb[:],
                                     qT_sb[:, 512 * ch:512 * ch + 512],
                                     start=True, stop=True)
                    sl = slice(512 * ch, 512 * ch + 512)
                    nc.vector.tensor_mul(u_sb[0:64, sl], ps[0:HD, :], csq_sb[0:64, sl])
                    nc.vector.tensor_mul(u_sb[64:128, sl], ps[0:HD, :], csq_sb[64:128, sl])

                # ---- S3: attention ----
                o_chunks = []
                for qc in range(NQC):
                    qsl = slice(512 * qc, 512 * qc + 512)
                    o_ps = pso.tile([HD + 1, 512], f32)
                    n_kt = 4 * qc + 4
                    for kt in range(n_kt):
                        s_ps = ps512.tile([128, 512], f32, tag='sps', bufs=2)
                        nc.tensor.matmul(s_ps[:], kT_sb[:, 128 * kt:128 * kt + 128],
                                         qT_sb[:, qsl], start=True, stop=False)
                        nc.tensor.matmul(s_ps[:], csk_sb[:, 128 * kt:128 * kt + 128],
                                         u_sb[:, qsl], start=False, stop=True)
                        m = kt - 4 * qc
                        if m > 0:
                            nc.vector.tensor_scalar_add(s_ps[:, 0:128 * m],
                                                        s_ps[:, 0:128 * m], -1e5)
                        if m >= 0:
                            msl = slice(128 * m, 128 * m + 128)
                            nc.vector.tensor_add(s_ps[:, msl], s_ps[:, msl], maskadd_sb[:])
                        p_sb = ptpool.tile([128, 512], f32r, tag="pT")
                        nc.scalar.activation(p_sb[:], s_ps[:],
                                             mybir.ActivationFunctionType.Exp,
                                             scale=0.125)
                        nc.tensor.matmul(o_ps[:], v_sb[kt][:], p_sb[:],
                                         start=(kt == 0), stop=(kt == n_kt - 1))
                    recip = smpool.tile([1, 512], f32, tag="recip")
                    nc.vector.reciprocal(recip[:], o_ps[HD:HD + 1, :])
                    bcast = smpool.tile([HD, 512], f32, tag="bcast")
                    nc.gpsimd.partition_broadcast(bcast[:], recip[:])
                    o_sb = ocpool.tile([HD, 512], f32r, tag="osb", name=f"o{b}_{qc}")
                    nc.vector.tensor_mul(o_sb[:], o_ps[0:HD, :], bcast[:])
                    o_chunks.append(o_sb)

                # ---- S4: row-parallel output Dense partials ----
                for mt in range(NDT):
                    for ch in range(NQC):
                        ps = ps512.tile([128, 512], f32, tag='ps', bufs=2)
                        nc.tensor.matmul(ps[:], outw_sb[:, 128 * mt:128 * mt + 128],
                                         o_chunks[ch][:], start=True, stop=True)
                        o2 = otpool.tile([128, 512], f32, tag="o2")
                        nc.vector.tensor_copy(o2[:], ps[:])
                        nc.sync.dma_start(
                            out=rs_in[4 * b + ch, 128 * mt:128 * mt + 128, :],
                            in_=o2[:])

            # ---- S5: ReduceScatter partials -> this core's seq slice ----
            tc.strict_bb_all_engine_barrier()
            nc.gpsimd.collective_compute(
                "ReduceScatter", mybir.AluOpType.add,
                replica_groups=GROUPS, ins=[rs_in[:]], outs=[rs_out[:]])
            tc.strict_bb_all_engine_barrier()

            # add bias, transpose to (s, x)-major, and find this core's absmax
            agt = [otpool.tile([128, X], f16, tag=f"agt{s2}", bufs=1,
                               name=f"agt{s2}")
                   for s2 in range(4)]
            mxp = otpool.tile([128, NDT], f32, tag="mxp", bufs=1)
            for mt in range(NDT):
                r_sb = otpool.tile([128, SC], f32, tag="rsb")
                nc.sync.dma_start(out=r_sb[:], in_=rs_out[128 * mt:128 * mt + 128, :])
                o3 = otpool.tile([128, SC], f16, tag="o3")
                nc.scalar.activation(o3[:], r_sb[:],
                                     mybir.ActivationFunctionType.Identity,
                                     bias=outb_sb[:, mt:mt + 1])
                nc.vector.tensor_reduce(mxp[:, mt:mt + 1], o3[:],
                                        axis=mybir.AxisListType.X,
                                        op=mybir.AluOpType.max,
                                        apply_absolute_value=True)
                tpp = ps512.tile([128, 512], f16, tag="tps", bufs=2,
                                 name=f"tpp{mt}")
                for s2 in range(4):
                    nc.tensor.transpose(tpp[:, 128 * s2:128 * s2 + 128],
                                        o3[:, 128 * s2:128 * s2 + 128],
                                        ident_h[:])
                for s2 in range(4):
                    nc.vector.tensor_copy(agt[s2][:, 128 * mt:128 * mt + 128],
                                          tpp[:, 128 * s2:128 * s2 + 128])
            # local absmax -> global absmax across cores
            mx1 = otpool.tile([128, 1], f32, tag="mx1", bufs=1)
            nc.vector.tensor_reduce(mx1[:, 0:1], mxp[:],
                                    axis=mybir.AxisListType.X,
                                    op=mybir.AluOpType.max)
            mxr = otpool.tile([128, 1], f32, tag="mxr", bufs=1)
            nc.gpsimd.partition_all_reduce(mxr[:], mx1[:], channels=128,
                                           reduce_op=bass_isa.ReduceOp.max)
            nc.sync.dma_start(out=mx_l[:], in_=mxr[0:1, 0:1])
            tc.strict_bb_all_engine_barrier()
            nc.gpsimd.collective_compute(
                "AllReduce", mybir.AluOpType.max,
                replica_groups=GROUPS, ins=[mx_l[:]], outs=[mx_g[:]])
            tc.strict_bb_all_engine_barrier()
            # scale = 127 / gmax, broadcast to all partitions
            gmax_sb = otpool.tile([1, 1], f32, tag="gmax", bufs=1)
            nc.sync.dma_start(out=gmax_sb[:], in_=mx_g[:])
            rcp = otpool.tile([1, 1], f32, tag="rcp", bufs=1)
            nc.vector.reciprocal(rcp[:], gmax_sb[:])
            nc.vector.tensor_scalar_mul(rcp[:], rcp[:], 127.0)
            scl = otpool.tile([128, 1], f32, tag="scl", bufs=1)
            nc.gpsimd.partition_broadcast(scl[:], rcp[:])
            # quantize this core's slice to int8 and gather everywhere
            for s2 in range(4):
                qt = otpool.tile([128, X], i8, tag="qt")
                nc.scalar.activation(qt[:], agt[s2][:],
                                     mybir.ActivationFunctionType.Identity,
                                     scale=scl[:, 0:1])
                nc.sync.dma_start(out=ag_in[128 * s2:128 * s2 + 128, :], in_=qt[:])
            tc.strict_bb_all_engine_barrier()
            nc.gpsimd.collective_compute(
                "AllGather", mybir.AluOpType.bypass,
                replica_groups=GROUPS, ins=[ag_in[:]], outs=[ag_out[:]])
            tc.strict_bb_all_engine_barrier()
            nc.sync.dma_start(out=out_q[0:B * S, :], in_=ag_out[:])
            nc.sync.dma_start(out=out_q[B * S:B * S + 1, 0:4],
                              in_=gmax_sb[:].bitcast(i8))

    nc.finalize()
    return nc


class _Runner:
    """Cached jitted shard_map executor for the Bass kernel.

    Mirrors bass2jax.run_bass_via_pjrt but (a) builds the jit once, (b) keeps
    input-independent constants committed on device, (c) materializes output
    buffers in-graph instead of shipping zeros from host.
    """

    CONST_NAMES = ("csq", "csk", "maskadd", "ident", "ones")

    def __init__(self):
        import jax
        import jax.numpy as jnp
        from jax.sharding import Mesh, PartitionSpec, NamedSharding
        from jax.experimental.shard_map import shard_map
        import concourse.mybir as mybir
        from concourse.bass2jax import (
            install_neuronx_cc_hook, partition_id_tensor, _bass_exec_p)

        install_neuronx_cc_hook()
        nc = _build()
        self.nc = nc

        partition_name = nc.partition_id_tensor.name if nc.partition_id_tensor else None
        in_names, out_names, out_avals = [], [], []
        for alloc in nc.m.functions[0].allocations:
            if not isinstance(alloc, mybir.MemoryLocationSet):
                continue
            name = alloc.memorylocations[0].name
            if alloc.kind == "ExternalInput":
                if name != partition_name:
                    in_names.append(name)
            elif alloc.kind == "ExternalOutput":
                out_names.append(name)
                out_avals.append(jax.core.ShapedArray(
                    tuple(alloc.tensor_shape), mybir.dt.np(alloc.dtype)))
        self.in_names = in_names
        self.out_names = out_names
        self.out_avals = out_avals
        in_names_all = in_names + out_names + ([partition_name] if partition_name else [])

        def _body(*args):
            operands = list(args)
            if partition_name is not None:
                operands.append(partition_id_tensor())
            outs = _bass_exec_p.bind(
                *operands,
                out_avals=tuple(out_avals),
                in_names=tuple(in_names_all),
                out_names=tuple(out_names),
                lowering_input_output_aliases=(),
                sim_require_finite=True,
                sim_require_nnan=True,
                nc=nc)
            return tuple(outs)

        devices = jax.devices()[:N_CORES]
        assert len(devices) == N_CORES
        mesh = Mesh(np.asarray(devices), ("core",))
        self.sharding = NamedSharding(mesh, PartitionSpec("core"))
        self.rep_sharding = NamedSharding(mesh, PartitionSpec())
        # out_f is identical on every core (device-side AllGather) ->
        # replicated: jax fetches a single contiguous shard.
        in_specs = (PartitionSpec("core"),) * len(in_names) + \
            (PartitionSpec(),) * len(out_names)
        out_specs = (PartitionSpec(),) * len(out_names)
        self.fn = jax.jit(shard_map(
            _body, mesh=mesh, in_specs=in_specs, out_specs=out_specs,
            check_rep=False))

        # Jitted CPU helpers: multithreaded pack (f32 -> f16 blob) and int8
        # dequant -- ~2x faster than single-threaded numpy casts.
        self._cpu = jax.devices("cpu")[0]

        def _pack(x, qkv, q_bias, positional, out_w, out_b):
            f16 = jnp.float16
            # 12-bit x: quantize with an f16-rounded step so the device
            # dequantizes with the bit-identical scale.
            xr = x.reshape(N_CORES, -1)
            step = jnp.maximum((jnp.max(jnp.abs(xr)) / 2047.0).astype(f16),
                               jnp.asarray(1e-7, f16))
            q = jnp.clip(jnp.round(xr / step.astype(jnp.float32)),
                         -2047, 2047).astype(jnp.int16)
            hi = jax.lax.bitcast_convert_type((q >> 4).astype(jnp.int8),
                                              jnp.uint8)
            lo = (q & 15).astype(jnp.uint8)
            nib = lo[:, 0::2] | (lo[:, 1::2] << 4)
            xbytes = jnp.concatenate([hi, nib], axis=1)
            x_slots = jax.lax.bitcast_convert_type(
                xbytes.reshape(N_CORES, -1, 2), f16)
            xstep = jnp.zeros((N_CORES, 64), f16).at[:, 0].set(step)
            wqk = qkv[:, 0:2].transpose(2, 0, 1, 3).reshape(N_CORES, -1).astype(f16)
            wv = qkv[:, 2].transpose(1, 0, 2).reshape(N_CORES, -1).astype(f16)
            posT = positional.transpose(1, 2, 0).reshape(N_CORES, -1).astype(f16)
            qb = q_bias.astype(f16)
            ow = out_w.reshape(N_CORES, -1).astype(f16)
            ob = jnp.broadcast_to(out_b.astype(f16)[None, :], (N_CORES, X))
            return jnp.concatenate([x_slots, xstep, wqk, wv, posT, qb, ow, ob],
                                   axis=1)

        def _dq(body, scale):
            return (body.astype(jnp.float32) * scale).reshape(B, S, X)

        with jax.default_device(self._cpu):
            self._pack_fn = jax.jit(_pack)
            self._dq_fn = jax.jit(_dq)

        # Commit input-independent constants to device once.
        csq, csk, maskadd, ident = _host_constants()
        const_global = {
            "csq": np.broadcast_to(csq, (N_CORES,) + csq.shape).reshape(N_CORES * 128, S),
            "csk": np.broadcast_to(csk, (N_CORES,) + csk.shape).reshape(N_CORES * 128, S),
            "maskadd": np.broadcast_to(maskadd, (N_CORES, 128, 128)).reshape(N_CORES * 128, 128),
            "ident": np.broadcast_to(ident, (N_CORES, 128, 128)).reshape(N_CORES * 128, 128),
            "ones": np.ones((N_CORES * 128, 1), np.float32),
        }
        import jax as _jax
        self.const_dev = {
            k: _jax.device_put(np.ascontiguousarray(v), self.sharding)
            for k, v in const_global.items()}
        # Dummy output-operand buffers, committed once (the kernel fully
        # overwrites every output, so their contents are irrelevant).
        self.zero_dev = [
            _jax.device_put(np.zeros(a.shape, a.dtype), self.rep_sharding)
            for a in out_avals]
        _jax.block_until_ready(list(self.const_dev.values()) + self.zero_dev)

    def __call__(self, named_globals):
        args = []
        for name in self.in_names:
            if name in self.const_dev:
                args.append(self.const_dev[name])
            else:
                args.append(named_globals[name])
        args.extend(self.zero_dev)
        outs = self.fn(*args)
        return dict(zip(self.out_names, (np.asarray(o) for o in outs)))


def _get_runner():
    if "runner" not in _CACHE:
        _CACHE["runner"] = _Runner()
    return _CACHE["runner"]


def kernel(x, qkv, q_bias, positional, out_w, out_b, _want_results=False, _trace=False):
    x = np.asarray(x, dtype=np.float32)
    qkv = np.asarray(qkv, dtype=np.float32)
    q_bias = np.asarray(q_bias, dtype=np.float32)
    positional = np.asarray(positional, dtype=np.float32)
    out_w = np.asarray(out_w, dtype=np.float32)
    out_b = np.asarray(out_b, dtype=np.float32)

    runner = _get_runner()
    import jax

    # One packed f16 blob per core (core c == head c == sequence slice c);
    # region order matches _PK_LAYOUT.
    with jax.default_device(runner._cpu):
        blob = np.asarray(
            runner._pack_fn(x, qkv, q_bias, positional, out_w, out_b))

    res = runner({"pk": blob.reshape(N_CORES * NPK, 1)})
    a = res["out_q"]
    gmax = a[B * S, 0:4].copy().view(np.float32)[0]
    with jax.default_device(runner._cpu):
        out = np.asarray(runner._dq_fn(a[:B * S], np.float32(gmax / 127.0)))
    if _want_results:
        class _R:
            exec_time_ns = None
            per_core_scope_times = None
            instructions_and_trace = None
        return out, _R()
    return out
